# revision 21
# baseline (speedup 1.0000x reference)
"""Trainium2 Bass kernel for nn_BL_36721970381090 (dense_mlp).

Math: the reference network
    item1 = einsum("td,bds->bts", W11, x)
    item2 = relu(einsum("bts,us->btu", item1, fc2_w) + bias1)
    item3 = einsum("ot,btu->bou", W12, item2)
    out   = (einsum("bou,pu->bop", item3, fc4_w) + bias2)[..., 0]
collapses (Kronecker identity) to a plain 2-layer MLP applied per batch row:
    out[b] = M2 @ relu(M1 @ vec(x[b]) + b1) + b2
with M1 = kron(W11, fc2_w) [600, 400], M2 = kron(W12, fc4_w) [3, 600],
b1 = bias1.reshape(600), b2 = bias2[:, 0].

Strategy: pure data parallel over 8 NeuronCores (batch split 131072 -> 8 x
16384). Host pre-transposes x to feature-major xT [400, Bc] per core and casts
to bf16 (input is the only large tensor; bf16 halves HBM traffic and doubles
PE throughput vs fp32's 2-pass matmul). On-chip: feature-major pipeline with
batch in the moving free dim - no on-chip transposes at all.
  layer1: psum[(t,u) chunk 120, b 512] += M1T_k[100,120].T @ xT_k[100,512]
          (4 K-chunks x 5 M-chunks)
  relu+bias1 on ScalarE (PSUM -> SBUF, cast to bf16)
  layer2: psum[3, b 512] += M2T_m[120,3].T @ relu_m[120,512]  (5 chunks)
  bias2 folded on host after gather.
"""

import numpy as np
import ml_dtypes
from contextlib import ExitStack

import concourse.bass as bass
import concourse.bacc as bacc
import concourse.mybir as mybir
from concourse.bass import ds
from concourse.tile import TileContext
from concourse.bass_utils import run_bass_kernel_spmd

B, D1, D2 = 131072, 40, 10
T0, T1, O0 = 120, 5, 3
NCORES = 8
BC = B // NCORES          # 16384 batch per core
KF = D1 * D2              # 400 input features (d, s)
MF = T0 * T1              # 600 hidden features (t, u)
KC = 100                  # K-chunk (4 chunks of 100 partitions)
MC = 120                  # M-chunk (5 chunks of 120 partitions)
NB = 512                  # matmul free-dim block (1 PSUM bank fp32)
NBD = 2048                # DMA block (4 x NB)

F32 = mybir.dt.float32
BF16 = mybir.dt.bfloat16
BF = ml_dtypes.bfloat16
RELU = mybir.ActivationFunctionType.Relu

_CACHE = {}


def _build_nc():
    nc = bacc.Bacc()
    xt = nc.dram_tensor("xt", (KF, BC), BF16, kind="ExternalInput")
    m1t = nc.dram_tensor("m1t", (KF, MF), BF16, kind="ExternalInput")
    m2t = nc.dram_tensor("m2t", (MC, 5 * O0), BF16, kind="ExternalInput")
    b1 = nc.dram_tensor("b1", (MC, 5), F32, kind="ExternalInput")
    outT = nc.dram_tensor("outT", (O0, BC), F32, kind="ExternalOutput")

    nk = KF // KC  # 4
    nm = MF // MC  # 5

    with TileContext(nc) as tc, ExitStack() as ctx:
        consts = ctx.enter_context(tc.tile_pool(name="consts", bufs=1))
        # layer-1 weights: one SBUF tile per K-chunk, [100, 600]
        m1_sb = [consts.tile([KC, MF], BF16, tag=f"m1_{k}", name=f"m1sb{k}") for k in range(nk)]
        for k in range(nk):
            nc.sync.dma_start(m1_sb[k][:, :], m1t[ds(k * KC, KC), :])
        # layer-2 weights: [120, 5*3], chunk m at cols [3m, 3m+3) - single DMA
        # (multiple writers into one tile would pile sync-waits on the reader)
        m2_sb = consts.tile([MC, nm * O0], BF16, tag="m2")
        nc.sync.dma_start(m2_sb[:, :], m2t[:, :])
        # bias1: [120, 5], col m = chunk m - single DMA
        b1_sb = consts.tile([MC, nm], F32, tag="b1")
        nc.sync.dma_start(b1_sb[:, :], b1[:, :])
        xpool = ctx.enter_context(tc.tile_pool(name="xp", bufs=3))
        opool = ctx.enter_context(tc.tile_pool(name="op", bufs=2))
        rpool = ctx.enter_context(tc.tile_pool(name="rp", bufs=8))
        ps1p = ctx.enter_context(tc.tile_pool(name="ps1", bufs=5, space="PSUM"))
        ps2p = ctx.enter_context(tc.tile_pool(name="ps2", bufs=2, space="PSUM"))

        for blk in range(BC // NBD):
            if blk == 0:
                # warmup block: per-512 tiles so PE starts after ~400KB of DMA
                x0 = [
                    [xpool.tile([KC, NB], BF16, tag=f"w{k}_{jj}", name=f"x0_{k}_{jj}")
                     for k in range(nk)]
                    for jj in range(NBD // NB)
                ]
                for jj in range(NBD // NB):
                    for k in range(nk):
                        nc.sync.dma_start(
                            x0[jj][k][:, :],
                            xt[ds(k * KC, KC), ds(jj * NB, NB)],
                        )
            else:
                xk = [xpool.tile([KC, NBD], BF16, tag=f"x{k}", name=f"xk{k}") for k in range(nk)]
                for k in range(nk):
                    nc.sync.dma_start(xk[k][:, :], xt[ds(k * KC, KC), ds(blk * NBD, NBD)])
            # layer 1 K-contiguous: each stationary (k,m) streams all 4 jj
            # blocks before switching -> 4x fewer LDWEIGHTS, denser PE stream.
            # Warmup block instead runs jj-outer so the first 20 MMs only need
            # jj=0's four small DMAs (PE starts ~2.5us in).
            rtiles = {}
            if blk == 0 or blk == BC // NBD - 1:
                # jj-outer at the pipeline edges: start needs only jj=0's
                # DMAs; end drains layer-2 incrementally per jj
                for jj in range(NBD // NB):
                    for m in range(nm):
                        ps = ps1p.tile([MC, NB], F32, tag="ps1", name=f"p0{m}{jj}")
                        for k in range(nk):
                            rhs = (x0[jj][k][:, :] if blk == 0
                                   else xk[k][:, ds(jj * NB, NB)])
                            nc.tensor.matmul(
                                ps[:, :], m1_sb[k][:, ds(m * MC, MC)],
                                rhs,
                                start=(k == 0), stop=(k == nk - 1),
                            )
                        r = rpool.tile([MC, NB], BF16, tag=f"r{m}", name=f"r0{m}{jj}")
                        nc.scalar.activation(r[:, :], ps[:, :], RELU,
                                             bias=b1_sb[:, ds(m, 1)])
                        rtiles[(m, jj)] = r
            else:
                for m in range(nm):
                    pss = []
                    for jj in range(NBD // NB):
                        ps = ps1p.tile([MC, NB], F32, tag="ps1", name=f"ps{m}{jj}")
                        pss.append(ps)
                    for k in range(nk):
                        lhs = m1_sb[k][:, ds(m * MC, MC)]
                        for jj in range(NBD // NB):
                            nc.tensor.matmul(
                                pss[jj][:, :], lhs, xk[k][:, ds(jj * NB, NB)],
                                start=(k == 0), stop=(k == nk - 1),
                            )
                    for jj in range(NBD // NB):
                        r = rpool.tile([MC, NB], BF16, tag=f"r{m}", name=f"rt{m}{jj}")
                        nc.scalar.activation(r[:, :], pss[jj][:, :], RELU,
                                             bias=b1_sb[:, ds(m, 1)])
                        rtiles[(m, jj)] = r
            osb = opool.tile([O0, NBD], F32, tag="osb")
            for jj in range(NBD // NB):
                ps2 = ps2p.tile([O0, NB], F32, tag="ps2", name=f"ps2{jj}")
                for m in range(nm):
                    nc.tensor.matmul(
                        ps2[:, :],
                        m2_sb[:, ds(m * O0, O0)],
                        rtiles[(m, jj)][:, :],
                        start=(m == 0),
                        stop=(m == nm - 1),
                    )
                nc.vector.tensor_copy(osb[:, ds(jj * NB, NB)], ps2[:, :])
            nc.sync.dma_start(outT[:, ds(blk * NBD, NBD)], osb[:, :])
    nc.finalize()
    return nc


def kernel(x, W11, fc2_w, bias1, W12, fc4_w, bias2, _trace=False):
    x = np.asarray(x, dtype=np.float32)
    M1 = np.kron(np.asarray(W11, np.float32), np.asarray(fc2_w, np.float32))
    M2 = np.kron(np.asarray(W12, np.float32), np.asarray(fc4_w, np.float32))
    b1v = np.ascontiguousarray(np.asarray(bias1, np.float32).reshape(5, MC).T)
    b2v = np.asarray(bias2, np.float32)[:, 0]

    m1t = np.ascontiguousarray(M1.T).astype(BF)          # [400, 600]
    m2t = np.ascontiguousarray(
        M2.T.reshape(5, MC, O0).transpose(1, 0, 2).reshape(MC, 5 * O0)
    ).astype(BF)                                          # [120, 15]

    if "nc" not in _CACHE:
        _CACHE["nc"] = _build_nc()
    nc = _CACHE["nc"]

    in_maps = []
    for c in range(NCORES):
        xs = x[c * BC : (c + 1) * BC]                     # [BC, 40, 10]
        xtc = xs.transpose(1, 2, 0).reshape(KF, BC).astype(BF)
        in_maps.append({"xt": xtc, "m1t": m1t, "m2t": m2t, "b1": b1v})

    res = run_bass_kernel_spmd(nc, in_maps, core_ids=list(range(NCORES)), trace=_trace)
    outs = [np.asarray(res.results[c]["outT"], np.float32) for c in range(NCORES)]
    full = np.concatenate(outs, axis=1).T + b2v[None, :]  # [B, 3]
    if _trace:
        kernel.last_exec_time_ns = res.exec_time_ns
    return full.astype(np.float32)
